# revision 33
# baseline (speedup 1.0000x reference)
"""Trainium2 Bass kernel for spherical deep GMM classifier (DGMMC).

Reference computation (B=8192, D=1024, C=128 classes, K=8 comps, N=C*K=1024):
    bw = clip(bandwidths, 1e-3, 100); a = 1/bw
    log_prob[b,n] = -0.5*(D*log(2pi) + D*log(bw[n]) + sq_dist[b,n]/bw[n])
    log_prob += log_softmax(weights.reshape(C,K),1).reshape(N)
    lse1[b,c]  = LSE_k(log_prob[b,c*K+k]) + log_softmax(priors)[c]
    out[b,c]   = lse1[b,c] - LSE_c(lse1[b,c])

Strategy: data-parallel over batch across 8 cores.  The device computes the
single dominant GEMM  dot[b,n] = x[b,:] @ (means * (1/bw))[n,:].T  and writes
dot back as fp16; 6 of the 8 128-dim contraction subtiles run as fp8-e4m3
DoubleRow matmuls, the other 2 as fp16 (full fp8 fails the 2e-2 gate:
2.07e-2 measured offline).  Everything else is O(B*N) or smaller and runs
on the host around the device call.

Perf structure (vs. the naive schedule):
  - Input DMA triggers are hoisted into the kernel-entry all-engine
    barrier's "arrive" slots on SP/ACT, so HBM traffic starts ~2.5us
    earlier (right after the fixed walrus engine preamble instead of after
    the const-init barrier).
  - A chain of dummy "warmup" matmuls reading uninitialized SBUF is
    hoisted the same way onto the PE queue: the HAM clock-ramp (~4us of
    sustained PE activity to reach 8/8) runs concurrently with the input
    load instead of serializing in front of the real stream.
  - Each means chunk is split into its two 512-column halves with one half
    on each HWDGE queue, so both queues feed the first b-tile's pass sweep
    concurrently (~330 GB/s aggregate).
  - x stripes ride the gpsimd software-DGE queue, keeping the two HWDGE
    queues free for means + output traffic.
  - The last b-tile is processed h-chunk-major and its output is cast +
    DMA'd per 512-column chunk, split across both queues by partition
    halves, to minimize the post-stream tail.
"""

import math

import numpy as np

B, D, C, K = 8192, 1024, 128, 8
N = C * K
NCORES = 8
BLOC = B // NCORES  # rows per core
P = 128
NSUB = D // P  # 128-row contraction subtiles
NH = N // 512
LOG_2PI = math.log(2.0 * math.pi)

_CACHE: dict = {}


def _build_nc(cfg=None):
    import concourse.bacc as bacc
    import concourse.bass as bass
    import concourse.mybir as mybir
    import concourse.tile as tile
    from concourse.tile import add_dep_helper

    defaults = dict(
        fp8_sub=6,       # leading 128-dim subtiles done in e4m3 DoubleRow
        warmup=8,        # dummy matmuls (garbage SBUF src) hoisted into the
                         # entry barrier to ramp the HAM clock to 8/8 while
                         # the input DMA head is in flight
        warmup_tail=2,   # non-hoisted warmups at stream top as a cushion so
                         # the PE never gaps between warmup chain and data
        hoist=True,      # move triggers/warmups into block-0 arrive slots
        xt_queue="pool", # "pool": x stripes on gpsimd SWDGE; "hw": on HWDGE
        bufs_work=4,
        psum_bufs=3,     # 3x [P,N] fp32 = 6 banks; wu_ps takes a 7th
        sem_stop=200,    # shrink kernel semaphore range (reset-sweep length)
        tail_stagger=True,
        tail_split=True, # split last-tile output DMAs across both queues
    )
    cfg = {**defaults, **(cfg or {})}

    f32 = mybir.dt.float32
    f16 = mybir.dt.float16
    f8 = mybir.dt.float8e4
    DR = mybir.MatmulPerfMode.DoubleRow
    ET = mybir.EngineType

    S8 = cfg["fp8_sub"]
    assert S8 % 2 == 0 and 0 <= S8 <= NSUB
    S16 = NSUB - S8
    NPAIR = S8 // 2

    orig_range = bass.get_kernel_semaphore_range
    if cfg["sem_stop"]:
        stop = cfg["sem_stop"]
        bass.get_kernel_semaphore_range = lambda: range(150, stop)
    try:
        nc = bacc.Bacc(None, target_bir_lowering=False)
    finally:
        bass.get_kernel_semaphore_range = orig_range

    NB = BLOC // P  # number of 128-row batch tiles per core

    # x is host-prepacked into per-b-tile stripes already in SBUF layout
    # [bt, p, bytes] with the fp8 subtiles' bytes followed by the fp16
    # subtiles' bytes, so each b-tile's stationary operand is one contiguous
    # full-bandwidth DMA; matmuls use bitcast views into the stripe.
    X8B = S8 * P  # fp8 bytes per stripe row
    XBYTES = X8B + S16 * P * 2
    u8 = mybir.dt.uint8
    xtc = nc.dram_tensor("xtc", [NB, P, XBYTES], u8, kind="ExternalInput")
    mt8t = mt16t = None
    if S8:
        # [pair, half, p, s, 512]: each (pair, half) chunk is one DMA with
        # 1KB-contiguous per-partition rows
        mt8t = nc.dram_tensor("mt8t", [NPAIR, 2, P, 2, 512], f8, kind="ExternalInput")
    if S16:
        mt16t = nc.dram_tensor("mt16t", [S16, 2, P, 512], f16, kind="ExternalInput")
    lp = nc.dram_tensor("lp", [BLOC, N], f16, kind="ExternalOutput")

    # dummy warmup operands: raw (non-tile) SBUF garbage + a dedicated PSUM
    # bank; no data deps, so the hoisted matmuls are free of semaphore waits.
    # matmul() auto-emits a paired InstLdweights, so capture the warmup
    # instructions by diffing the entry block around the emission.
    wu_src = nc.alloc_sbuf_tensor("wu_src", [P, 512], f16)
    wu_ps = nc.alloc_psum_tensor("wu_ps", [P, 512], f32)
    mainblk = nc.m.functions[0].blocks[0]
    n_before = len(mainblk.instructions)
    for _ in range(cfg["warmup"]):
        nc.tensor.matmul(
            wu_ps[:], wu_src[:, 0:P], wu_src[:], start=True, stop=True
        )
    wu_insts = list(mainblk.instructions[n_before:])

    trig = {"sync": [], "scalar": [], "pool": []}

    with tile.TileContext(nc) as tc:
        with (
            tc.tile_pool(name="resident", bufs=1) as resident,
            tc.tile_pool(name="work", bufs=cfg["bufs_work"]) as work,
            tc.tile_pool(name="psum", bufs=cfg["psum_bufs"], space="PSUM") as psum_pool,
        ):
            xtc_sb = resident.tile([P, NB, XBYTES], u8, name="xtc_sb")
            # h-major SBUF layout: chunk (c|s, h) is contiguous within each
            # partition, so its load is one >=1KB-row DMA per partition
            mt8_sb = (
                resident.tile([P, NH, S8, 512], f8, name="mt8_sb") if S8 else None
            )
            mt16_sb = (
                resident.tile([P, NH, S16, 512], f16, name="mt16_sb") if S16 else None
            )

            def x8_view(bt, c):  # DR pass c stationary operand [P, 2, 128]
                return (
                    xtc_sb[:, bt, 0:X8B]
                    .bitcast(f8)
                    .rearrange("p (s c) -> p s c", c=P)[:, 2 * c : 2 * c + 2, :]
                )

            def x16_view(bt, s):  # fp16 pass s stationary operand [P, 128]
                return (
                    xtc_sb[:, bt, X8B:XBYTES]
                    .bitcast(f16)
                    .rearrange("p (s c) -> p s c", c=P)[:, s, :]
                )

            # extra warmups that stay at the top of the PE stream: if the
            # input DMA head outlasts the hoisted warmup chain these bridge
            # the gap so HAM doesn't re-throttle
            for _ in range(cfg["warmup_tail"]):
                nc.tensor.matmul(
                    wu_ps[:], wu_src[:, 0:P], wu_src[:], start=True, stop=True
                )

            # Input loads.  xt0 leads the sync queue; mt chunk halves are
            # hand-assigned across the two HWDGE queues so each chunk lands
            # just before the h-major b-tile-0 sweep consumes it; the other
            # x stripes ride the gpsimd software-DGE queue.
            def src_of(kind, idx, h):
                if kind == "p":
                    return mt8_sb[:, h, 2 * idx : 2 * idx + 2, :], mt8t[idx, h]
                return mt16_sb[:, h, idx, :], mt16t[idx, h]

            if S8 == 6 and S16 == 2:
                sync_order = [("x", 0, 0), ("p", 0, 0), ("p", 2, 0), ("s", 1, 0), ("s", 0, 1), ("p", 2, 1)]
                scalar_order = [("x", 1, 0), ("p", 0, 1), ("p", 1, 0), ("s", 0, 0), ("p", 1, 1), ("s", 1, 1)]
                pool_pre = []
            else:
                sync_order = [("x", 0, 0)] + [("p", c, 0) for c in range(NPAIR)] + [
                    ("s", s, 0) for s in range(S16)
                ] + [("x", 1, 0)]
                scalar_order = [("p", c, 1) for c in range(NPAIR)] + [
                    ("s", s, 1) for s in range(S16)
                ]
                pool_pre = []
            last_hw_xt = None
            for eng_name, eng, order in (
                ("sync", nc.sync, sync_order),
                ("scalar", nc.scalar, scalar_order),
            ):
                for kind, idx, h in order:
                    if kind == "x":
                        ti = eng.dma_start(xtc_sb[:, idx], xtc[idx])
                        last_hw_xt = ti
                    else:
                        dst, src = src_of(kind, idx, h)
                        ti = eng.dma_start(dst, src)
                    trig[eng_name].append(ti.ins)
            # Remaining x stripes ride the gpsimd SWDGE queue, but only
            # after the HWDGE-side critical loads have landed — otherwise
            # the software queue steals HBM bandwidth from the mt chunks
            # the b-tile-0 sweep is about to consume.
            xt_eng = {"pool": nc.gpsimd, "sync": nc.sync, "scalar": nc.scalar}[
                cfg["xt_queue"]
            ]
            xt_key = {"pool": "pool", "sync": "sync", "scalar": "scalar"}[
                cfg["xt_queue"]
            ]
            for kind, idx, h in pool_pre:
                dst, src = src_of(kind, idx, h)
                ti = nc.gpsimd.dma_start(dst, src)
                trig["pool"].append(ti.ins)
            for bt in range(2, NB):
                ti = xt_eng.dma_start(xtc_sb[:, bt], xtc[bt])
                if bt == 2 and xt_key == "pool":
                    add_dep_helper(
                        ti.ins,
                        last_hw_xt.ins,
                        sync=True,
                        reason="defer SWDGE xt behind critical HWDGE loads",
                    )
                trig[xt_key].append(ti.ins)

            # matmul pass list: DoubleRow fp8 pairs first, fp16 after
            def passes(bt):
                out = []
                for c in range(NPAIR):
                    out.append(
                        lambda h, c=c, bt=bt: (
                            x8_view(bt, c),
                            mt8_sb[:, h, 2 * c : 2 * c + 2, :],
                            DR,
                        )
                    )
                for s in range(S16):
                    out.append(
                        lambda h, s=s, bt=bt: (
                            x16_view(bt, s),
                            mt16_sb[:, h, s, :],
                            None,
                        )
                    )
                return out

            prev_mm = None

            def emit_out(bt, ot, h=None):
                # split every output across both HWDGE queues by partition
                # halves so no single queue ever backs up at the tail
                half = P // 2
                csl = slice(0, N) if h is None else slice(h * 512, (h + 1) * 512)
                nc.sync.dma_start(lp[bt * P : bt * P + half, csl], ot[0:half])
                nc.scalar.dma_start(
                    lp[bt * P + half : (bt + 1) * P, csl], ot[half:P]
                )

            # b-tiles 0+1 run as one fused pass-major group: the mt chunk
            # demand rate is halved (one chunk per 2 matmuls) while the
            # input DMAs are still landing, so the PE never gaps (a gap
            # >~1us makes HAM re-throttle the clock to 4/8 mid-stream)
            ps01 = [
                psum_pool.tile([P, N], f32, tag="ps", name=f"ps0{b}")
                for b in range(2)
            ]
            pl01 = [passes(0), passes(1)]
            npass = len(pl01[0])
            for h in range(NH):
                for i in range(npass):
                    for b in range(2):
                        lhsT, rhs, pm = pl01[b][i](h)
                        mmi = nc.tensor.matmul(
                            ps01[b][:, h * 512 : (h + 1) * 512],
                            lhsT,
                            rhs,
                            start=(i == 0),
                            stop=(i == npass - 1),
                            perf_mode=pm,
                        )
                        if i == npass - 1 and b == 1:
                            prev_mm = mmi
            for b in range(2):
                ot = work.tile([P, N], f16, tag="ot", name=f"ot0{b}")
                nc.vector.tensor_copy(ot, ps01[b])
                emit_out(b, ot)

            for bt in range(2, NB):
                stag = cfg["tail_stagger"] and bt == NB - 1
                if stag:
                    # separate per-h psum tiles so h0's cast (a tile-level
                    # read) doesn't falsely serialize against h1's matmuls
                    ps_h = [
                        psum_pool.tile([P, 512], f32, tag="ps", name=f"pst{h}")
                        for h in range(NH)
                    ]
                else:
                    ps = psum_pool.tile([P, N], f32, tag="ps")
                pl = passes(bt)
                npass = len(pl)
                # h-chunk-major: on the last tile h0's cast+DMA overlap h1's
                # matmuls.  NOTE: pass order within an accumulation group
                # must keep all DoubleRow passes before all fp16 passes —
                # mixing them non-monotonically corrupts the accumulation.
                for h in range(NH):
                    for i in range(npass):
                        lhsT, rhs, pm = pl[i](h)
                        pdst = (
                            ps_h[h][:, :] if stag else ps[:, h * 512 : (h + 1) * 512]
                        )
                        mmi = nc.tensor.matmul(
                            pdst,
                            lhsT,
                            rhs,
                            start=(i == 0),
                            stop=(i == npass - 1),
                            perf_mode=pm,
                        )
                        # serialize b-tile groups on PE so each group
                        # completes (and its copy-out starts) ASAP
                        if h == 0 and i == 0 and prev_mm is not None:
                            add_dep_helper(
                                mmi.ins,
                                prev_mm.ins,
                                sync=False,
                                reason="group-sequential PE order",
                            )
                        if i == npass - 1:
                            prev_mm = mmi
                            if stag:
                                ot = work.tile([P, 512], f16, tag="ot2")
                                nc.vector.tensor_copy(ot, ps_h[h][:, :])
                                emit_out(bt, ot, h=h)
                if not stag:
                    ot = work.tile([P, N], f16, tag="ot")
                    nc.vector.tensor_copy(ot, ps)
                    emit_out(bt, ot)

    if cfg["hoist"]:
        _hoist(nc, mybir, trig, wu_insts)

    nc.compile()
    return nc


def _hoist(nc, mybir, trig, wu_insts):
    """Move the input DMA triggers and warmup matmuls from the tile-context
    block into the entry block's all-engine-barrier arrive slots.

    The entry barrier is, per engine, (InstDrain[arrive], InstEventSemaphore
    [wait-release]); instructions placed between the two run right after that
    engine's fixed walrus preamble without delaying any other engine.  The
    gpsimd x-stripe triggers go after the barrier release (gpsimd is the
    barrier master, so anything before its release EventSemaphore would
    stall every engine)."""
    ET = mybir.EngineType
    f = nc.m.functions[0]
    b0, b1 = f.blocks[0], f.blocks[1]

    moved = {
        ET.SP: list(trig["sync"]),
        ET.Activation: list(trig["scalar"]),
        ET.PE: list(wu_insts),
        ET.Pool: list(trig["pool"]),
    }
    # warmups emitted pre-tile-context already live in b0 (after the
    # barrier); everything else is in b1
    move_ids = {id(x) for insts in moved.values() for x in insts}
    b0.instructions = [x for x in b0.instructions if id(x) not in move_ids]
    b1.instructions = [x for x in b1.instructions if id(x) not in move_ids]

    def arrive_slot(eng):
        for i, ins in enumerate(b0.instructions):
            if isinstance(ins, mybir.InstDrain) and ins.engine == eng:
                return i + 1
        raise RuntimeError(f"no barrier drain found for {eng}")

    def after_release():
        last = None
        for i, ins in enumerate(b0.instructions):
            if isinstance(ins, mybir.InstEventSemaphore) and ins.engine == ET.Pool:
                last = i
        assert last is not None
        return last + 1

    for eng in (ET.SP, ET.Activation, ET.PE):
        if moved[eng]:
            pos = arrive_slot(eng)
            b0.instructions[pos:pos] = moved[eng]
    if moved[ET.Pool]:
        pos = after_release()
        b0.instructions[pos:pos] = moved[ET.Pool]


def _host_prep(x, means, bandwidths, weights, priors, fp8_sub):
    """Pack transposed GEMM operands; compute host-side affine terms."""
    import ml_dtypes

    x = np.asarray(x, dtype=np.float32)
    means = np.asarray(means, dtype=np.float32)

    bw = np.clip(np.asarray(bandwidths, dtype=np.float64), 0.001, 100.0)
    a = 1.0 / bw
    m_sq = np.einsum(
        "nd,nd->n", means.astype(np.float64), means.astype(np.float64)
    )
    w = np.asarray(weights, dtype=np.float64).reshape(C, K)
    log_w = (
        w
        - np.log(np.exp(w - w.max(1, keepdims=True)).sum(1, keepdims=True))
        - w.max(1, keepdims=True)
    ).reshape(N)
    pr = np.asarray(priors, dtype=np.float64)
    log_pri = pr - (np.log(np.exp(pr - pr.max()).sum()) + pr.max())
    cvec = (
        -0.5 * (D * LOG_2PI + D * np.log(bw) + m_sq * a)
        + log_w
        + np.repeat(log_pri, K)
    )
    ah = -0.5 * a
    xsq = np.einsum("bd,bd->b", x.astype(np.float64), x.astype(np.float64))

    # pack x into per-core, per-b-tile stripes [core, bt, p(row), bytes]:
    # fp8 subtile bytes then fp16 subtile bytes, matching the device bitcast
    nbt = BLOC // P
    ds = fp8_sub * P
    xt_t = x.T  # [D, B]
    mt_t = means.T * a  # [D, N]

    def pack_x(arr, dt):  # arr [d, B] -> [core, bt, p(row), sub*col] bytes
        sub = arr.shape[0] // P
        packed = np.ascontiguousarray(
            arr.astype(dt).reshape(sub, P, NCORES, nbt, P).transpose(2, 3, 1, 0, 4)
        )
        return packed.reshape(NCORES, nbt, P, -1).view(np.uint8)

    chunks = []
    if fp8_sub:
        chunks.append(pack_x(xt_t[:ds], ml_dtypes.float8_e4m3))
    if ds < D:
        chunks.append(pack_x(xt_t[ds:], np.float16))
    parts = {"xtc": np.concatenate(chunks, axis=3)}
    if fp8_sub:
        # [pair, half, p, s(2), 512]: element [c,h,p,s,j] = mt[(2c+s)*P+p,
        # h*512+j]
        m8 = mt_t[:ds].astype(ml_dtypes.float8_e4m3)
        m8 = m8.reshape(fp8_sub // 2, 2, P, 2, 512).transpose(0, 3, 2, 1, 4)
        parts["mt8t"] = np.ascontiguousarray(m8)
    if ds < D:
        m16 = mt_t[ds:].astype(np.float16)
        m16 = m16.reshape((D - ds) // P, P, 2, 512).transpose(0, 2, 1, 3)
        parts["mt16t"] = np.ascontiguousarray(m16)
    return parts, cvec, ah, xsq


def _host_finish(lp, cvec, ah, xsq):
    """lp: [B, N] fp16 device GEMM result -> [B, C] float32 log-mixture."""
    logp = lp.astype(np.float32)
    logp += cvec.astype(np.float32)[None, :]
    logp += xsq.astype(np.float32)[:, None] * ah.astype(np.float32)[None, :]
    v = logp.reshape(B, C, K)
    m = v.max(2)
    lse1 = m + np.log(np.exp(v - m[:, :, None]).sum(2, dtype=np.float32))
    z = lse1.max(1, keepdims=True)
    out = lse1 - (
        z + np.log(np.exp(lse1 - z).sum(1, keepdims=True, dtype=np.float32))
    )
    return out.astype(np.float32)


def _run(x, means, bandwidths, weights, priors, trace=False, cfg=None):
    from concourse.bass_utils import run_bass_kernel_spmd

    key = tuple(sorted((cfg or {}).items()))
    if key not in _CACHE:
        _CACHE[key] = _build_nc(cfg)
    nc = _CACHE[key]
    fp8_sub = (cfg or {}).get("fp8_sub", 6)

    parts, cvec, ah, xsq = _host_prep(
        x, means, bandwidths, weights, priors, fp8_sub
    )
    in_maps = [
        {
            k: np.ascontiguousarray(v[i]) if k.startswith("xt") else v
            for k, v in parts.items()
        }
        for i in range(NCORES)
    ]
    res = run_bass_kernel_spmd(nc, in_maps, core_ids=list(range(NCORES)), trace=trace)
    lp = np.concatenate([r["lp"] for r in res.results], axis=0)
    out = _host_finish(lp, cvec, ah, xsq)
    return out, res


def kernel(x, means, bandwidths, weights, priors):
    out, _ = _run(x, means, bandwidths, weights, priors, trace=False)
    return out


# revision 35
# speedup vs baseline: 1.0512x; 1.0512x over previous
"""Trainium2 Bass kernel for spherical deep GMM classifier (DGMMC).

Reference computation (B=8192, D=1024, C=128 classes, K=8 comps, N=C*K=1024):
    bw = clip(bandwidths, 1e-3, 100); a = 1/bw
    log_prob[b,n] = -0.5*(D*log(2pi) + D*log(bw[n]) + sq_dist[b,n]/bw[n])
    log_prob += log_softmax(weights.reshape(C,K),1).reshape(N)
    lse1[b,c]  = LSE_k(log_prob[b,c*K+k]) + log_softmax(priors)[c]
    out[b,c]   = lse1[b,c] - LSE_c(lse1[b,c])

Strategy: data-parallel over batch across 8 cores.  The device computes the
single dominant GEMM  dot[b,n] = x[b,:] @ (means * (1/bw))[n,:].T  and writes
dot back as fp16; 6 of the 8 128-dim contraction subtiles run as fp8-e4m3
DoubleRow matmuls, the other 2 as fp16 (full fp8 fails the 2e-2 gate:
2.07e-2 measured offline).  Everything else is O(B*N) or smaller and runs
on the host around the device call.

Perf structure (vs. the naive schedule):
  - Input DMA triggers are hoisted into the kernel-entry all-engine
    barrier's "arrive" slots on SP/ACT, so HBM traffic starts ~2.5us
    earlier (right after the fixed walrus engine preamble instead of after
    the const-init barrier).
  - A chain of dummy "warmup" matmuls reading uninitialized SBUF is
    hoisted the same way onto the PE queue: the HAM clock-ramp (~4us of
    sustained PE activity to reach 8/8) runs concurrently with the input
    load instead of serializing in front of the real stream.
  - Each means chunk is split into its two 512-column halves with one half
    on each HWDGE queue, so both queues feed the first b-tile's pass sweep
    concurrently (~330 GB/s aggregate).
  - x stripes ride the gpsimd software-DGE queue, keeping the two HWDGE
    queues free for means + output traffic.
  - The last b-tile is processed h-chunk-major and its output is cast +
    DMA'd per 512-column chunk, split across both queues by partition
    halves, to minimize the post-stream tail.
"""

import math

import numpy as np

B, D, C, K = 8192, 1024, 128, 8
N = C * K
NCORES = 8
BLOC = B // NCORES  # rows per core
P = 128
NSUB = D // P  # 128-row contraction subtiles
NH = N // 512
LOG_2PI = math.log(2.0 * math.pi)

_CACHE: dict = {}


def _build_nc(cfg=None):
    import concourse.bacc as bacc
    import concourse.bass as bass
    import concourse.mybir as mybir
    import concourse.tile as tile
    from concourse.tile import add_dep_helper

    defaults = dict(
        fp8_sub=6,       # leading 128-dim subtiles done in e4m3 DoubleRow
        warmup=8,        # dummy matmuls (garbage SBUF src) hoisted into the
                         # entry barrier to ramp the HAM clock to 8/8 while
                         # the input DMA head is in flight
        warmup_tail=2,   # non-hoisted warmups at stream top as a cushion so
                         # the PE never gaps between warmup chain and data
        hoist=True,      # move triggers/warmups into block-0 arrive slots
        xt_queue="pool", # "pool": x stripes on gpsimd SWDGE; "hw": on HWDGE
        bufs_work=4,
        psum_bufs=3,     # 3x [P,N] fp32 = 6 banks; wu_ps takes a 7th
        sem_stop=200,    # shrink kernel semaphore range (reset-sweep length)
        tail_stagger=True,
        tail_split=True, # split last-tile output DMAs across both queues
    )
    cfg = {**defaults, **(cfg or {})}

    f32 = mybir.dt.float32
    f16 = mybir.dt.float16
    f8 = mybir.dt.float8e4
    DR = mybir.MatmulPerfMode.DoubleRow
    ET = mybir.EngineType

    S8 = cfg["fp8_sub"]
    assert S8 % 2 == 0 and 0 <= S8 <= NSUB
    S16 = NSUB - S8
    NPAIR = S8 // 2

    orig_range = bass.get_kernel_semaphore_range
    if cfg["sem_stop"]:
        stop = cfg["sem_stop"]
        bass.get_kernel_semaphore_range = lambda: range(150, stop)
    try:
        nc = bacc.Bacc(None, target_bir_lowering=False)
    finally:
        bass.get_kernel_semaphore_range = orig_range

    NB = BLOC // P  # number of 128-row batch tiles per core

    # x is host-prepacked into per-b-tile stripes already in SBUF layout
    # [bt, p, bytes] with the fp8 subtiles' bytes followed by the fp16
    # subtiles' bytes, so each b-tile's stationary operand is one contiguous
    # full-bandwidth DMA; matmuls use bitcast views into the stripe.
    X8B = S8 * P  # fp8 bytes per stripe row
    XBYTES = X8B + S16 * P * 2
    u8 = mybir.dt.uint8
    xtc = nc.dram_tensor("xtc", [NB, P, XBYTES], u8, kind="ExternalInput")
    mt8t = mt16t = None
    if S8:
        # [pair, half, p, s, 512]: each (pair, half) chunk is one DMA with
        # 1KB-contiguous per-partition rows
        mt8t = nc.dram_tensor("mt8t", [NPAIR, 2, P, 2, 512], f8, kind="ExternalInput")
    if S16:
        mt16t = nc.dram_tensor("mt16t", [S16, 2, P, 512], f16, kind="ExternalInput")
    lp = nc.dram_tensor("lp", [BLOC, N], f16, kind="ExternalOutput")

    # dummy warmup operands: raw (non-tile) SBUF garbage + a dedicated PSUM
    # bank; no data deps, so the hoisted matmuls are free of semaphore waits.
    # matmul() auto-emits a paired InstLdweights, so capture the warmup
    # instructions by diffing the entry block around the emission.
    wu_src = nc.alloc_sbuf_tensor("wu_src", [P, 512], f16)
    wu_ps = nc.alloc_psum_tensor("wu_ps", [P, 512], f32)
    mainblk = nc.m.functions[0].blocks[0]
    n_before = len(mainblk.instructions)
    for _ in range(cfg["warmup"]):
        nc.tensor.matmul(
            wu_ps[:], wu_src[:, 0:P], wu_src[:], start=True, stop=True
        )
    wu_insts = list(mainblk.instructions[n_before:])

    trig = {"sync": [], "scalar": [], "pool": []}

    with tile.TileContext(nc) as tc:
        with (
            tc.tile_pool(name="resident", bufs=1) as resident,
            tc.tile_pool(name="work", bufs=cfg["bufs_work"]) as work,
            tc.tile_pool(name="psum", bufs=cfg["psum_bufs"], space="PSUM") as psum_pool,
        ):
            xtc_sb = resident.tile([P, NB, XBYTES], u8, name="xtc_sb")
            # h-major SBUF layout: chunk (c|s, h) is contiguous within each
            # partition, so its load is one >=1KB-row DMA per partition
            mt8_sb = (
                resident.tile([P, NH, S8, 512], f8, name="mt8_sb") if S8 else None
            )
            mt16_sb = (
                resident.tile([P, NH, S16, 512], f16, name="mt16_sb") if S16 else None
            )

            def x8_view(bt, c):  # DR pass c stationary operand [P, 2, 128]
                return (
                    xtc_sb[:, bt, 0:X8B]
                    .bitcast(f8)
                    .rearrange("p (s c) -> p s c", c=P)[:, 2 * c : 2 * c + 2, :]
                )

            def x16_view(bt, s):  # fp16 pass s stationary operand [P, 128]
                return (
                    xtc_sb[:, bt, X8B:XBYTES]
                    .bitcast(f16)
                    .rearrange("p (s c) -> p s c", c=P)[:, s, :]
                )

            # extra warmups that stay at the top of the PE stream: if the
            # input DMA head outlasts the hoisted warmup chain these bridge
            # the gap so HAM doesn't re-throttle
            for _ in range(cfg["warmup_tail"]):
                nc.tensor.matmul(
                    wu_ps[:], wu_src[:, 0:P], wu_src[:], start=True, stop=True
                )

            # Input loads.  xt0 leads the sync queue; mt chunk halves are
            # hand-assigned across the two HWDGE queues so each chunk lands
            # just before the h-major b-tile-0 sweep consumes it; the other
            # x stripes ride the gpsimd software-DGE queue.
            def src_of(kind, idx, h):
                if kind == "p":
                    return mt8_sb[:, h, 2 * idx : 2 * idx + 2, :], mt8t[idx, h]
                return mt16_sb[:, h, idx, :], mt16t[idx, h]

            if S8 == 6 and S16 == 2:
                sync_order = [("x", 0, 0), ("p", 0, 0), ("p", 2, 0), ("s", 1, 0), ("s", 0, 1), ("p", 2, 1)]
                scalar_order = [("x", 1, 0), ("p", 0, 1), ("p", 1, 0), ("s", 0, 0), ("p", 1, 1), ("s", 1, 1)]
                pool_pre = []
            else:
                sync_order = [("x", 0, 0)] + [("p", c, 0) for c in range(NPAIR)] + [
                    ("s", s, 0) for s in range(S16)
                ] + [("x", 1, 0)]
                scalar_order = [("p", c, 1) for c in range(NPAIR)] + [
                    ("s", s, 1) for s in range(S16)
                ]
                pool_pre = []
            last_mt_trig = None
            for eng_name, eng, order in (
                ("sync", nc.sync, sync_order),
                ("scalar", nc.scalar, scalar_order),
            ):
                for kind, idx, h in order:
                    if kind == "x":
                        ti = eng.dma_start(xtc_sb[:, idx], xtc[idx])
                    else:
                        dst, src = src_of(kind, idx, h)
                        ti = eng.dma_start(dst, src)
                        last_mt_trig = ti
                    trig[eng_name].append(ti.ins)
            # Remaining x stripes ride the gpsimd SWDGE queue, but only
            # after the HWDGE-side critical loads have landed — otherwise
            # the software queue steals HBM bandwidth from the mt chunks
            # the b-tile-0 sweep is about to consume.
            xt_eng = {"pool": nc.gpsimd, "sync": nc.sync, "scalar": nc.scalar}[
                cfg["xt_queue"]
            ]
            xt_key = {"pool": "pool", "sync": "sync", "scalar": "scalar"}[
                cfg["xt_queue"]
            ]
            for kind, idx, h in pool_pre:
                dst, src = src_of(kind, idx, h)
                ti = nc.gpsimd.dma_start(dst, src)
                trig["pool"].append(ti.ins)
            for bt in range(2, NB):
                ti = xt_eng.dma_start(xtc_sb[:, bt], xtc[bt])
                if bt == 2 and xt_key == "pool":
                    add_dep_helper(
                        ti.ins,
                        last_mt_trig.ins,
                        sync=True,
                        reason="defer SWDGE xt behind critical HWDGE loads",
                    )
                trig[xt_key].append(ti.ins)

            # matmul pass list: DoubleRow fp8 pairs first, fp16 after
            def passes(bt):
                out = []
                for c in range(NPAIR):
                    out.append(
                        lambda h, c=c, bt=bt: (
                            x8_view(bt, c),
                            mt8_sb[:, h, 2 * c : 2 * c + 2, :],
                            DR,
                        )
                    )
                for s in range(S16):
                    out.append(
                        lambda h, s=s, bt=bt: (
                            x16_view(bt, s),
                            mt16_sb[:, h, s, :],
                            None,
                        )
                    )
                return out

            prev_mm = None

            def emit_out(bt, ot, h=None):
                # split every output across both HWDGE queues by partition
                # halves so no single queue ever backs up at the tail
                half = P // 2
                csl = slice(0, N) if h is None else slice(h * 512, (h + 1) * 512)
                nc.sync.dma_start(lp[bt * P : bt * P + half, csl], ot[0:half])
                nc.scalar.dma_start(
                    lp[bt * P + half : (bt + 1) * P, csl], ot[half:P]
                )

            # b-tiles 0+1 run as one fused pass-major group: the mt chunk
            # demand rate is halved (one chunk per 2 matmuls) while the
            # input DMAs are still landing, so the PE never gaps (a gap
            # >~1us makes HAM re-throttle the clock to 4/8 mid-stream)
            ps01 = [
                psum_pool.tile([P, N], f32, tag="ps", name=f"ps0{b}")
                for b in range(2)
            ]
            pl01 = [passes(0), passes(1)]
            npass = len(pl01[0])
            for h in range(NH):
                for i in range(npass):
                    for b in range(2):
                        lhsT, rhs, pm = pl01[b][i](h)
                        mmi = nc.tensor.matmul(
                            ps01[b][:, h * 512 : (h + 1) * 512],
                            lhsT,
                            rhs,
                            start=(i == 0),
                            stop=(i == npass - 1),
                            perf_mode=pm,
                        )
                        if i == npass - 1 and b == 1:
                            prev_mm = mmi
            for b in range(2):
                ot = work.tile([P, N], f16, tag="ot", name=f"ot0{b}")
                nc.vector.tensor_copy(ot, ps01[b])
                emit_out(b, ot)

            for bt in range(2, NB):
                stag = cfg["tail_stagger"] and bt == NB - 1
                if stag:
                    # separate per-h psum tiles so h0's cast (a tile-level
                    # read) doesn't falsely serialize against h1's matmuls
                    ps_h = [
                        psum_pool.tile([P, 512], f32, tag="ps", name=f"pst{h}")
                        for h in range(NH)
                    ]
                else:
                    ps = psum_pool.tile([P, N], f32, tag="ps")
                pl = passes(bt)
                npass = len(pl)
                # h-chunk-major: on the last tile h0's cast+DMA overlap h1's
                # matmuls.  NOTE: pass order within an accumulation group
                # must keep all DoubleRow passes before all fp16 passes —
                # mixing them non-monotonically corrupts the accumulation.
                for h in range(NH):
                    for i in range(npass):
                        lhsT, rhs, pm = pl[i](h)
                        pdst = (
                            ps_h[h][:, :] if stag else ps[:, h * 512 : (h + 1) * 512]
                        )
                        mmi = nc.tensor.matmul(
                            pdst,
                            lhsT,
                            rhs,
                            start=(i == 0),
                            stop=(i == npass - 1),
                            perf_mode=pm,
                        )
                        # serialize b-tile groups on PE so each group
                        # completes (and its copy-out starts) ASAP
                        if h == 0 and i == 0 and prev_mm is not None:
                            add_dep_helper(
                                mmi.ins,
                                prev_mm.ins,
                                sync=False,
                                reason="group-sequential PE order",
                            )
                        if i == npass - 1:
                            prev_mm = mmi
                            if stag:
                                ot = work.tile([P, 512], f16, tag="ot2")
                                nc.vector.tensor_copy(ot, ps_h[h][:, :])
                                emit_out(bt, ot, h=h)
                if not stag:
                    ot = work.tile([P, N], f16, tag="ot")
                    nc.vector.tensor_copy(ot, ps)
                    emit_out(bt, ot)

    if cfg["hoist"]:
        _hoist(nc, mybir, trig, wu_insts)

    nc.compile()
    return nc


def _hoist(nc, mybir, trig, wu_insts):
    """Move the input DMA triggers and warmup matmuls from the tile-context
    block into the entry block's all-engine-barrier arrive slots.

    The entry barrier is, per engine, (InstDrain[arrive], InstEventSemaphore
    [wait-release]); instructions placed between the two run right after that
    engine's fixed walrus preamble without delaying any other engine.  The
    gpsimd x-stripe triggers go after the barrier release (gpsimd is the
    barrier master, so anything before its release EventSemaphore would
    stall every engine)."""
    ET = mybir.EngineType
    f = nc.m.functions[0]
    b0, b1 = f.blocks[0], f.blocks[1]

    moved = {
        ET.SP: list(trig["sync"]),
        ET.Activation: list(trig["scalar"]),
        ET.PE: list(wu_insts),
        ET.Pool: list(trig["pool"]),
    }
    # warmups emitted pre-tile-context already live in b0 (after the
    # barrier); everything else is in b1
    move_ids = {id(x) for insts in moved.values() for x in insts}
    b0.instructions = [x for x in b0.instructions if id(x) not in move_ids]
    b1.instructions = [x for x in b1.instructions if id(x) not in move_ids]

    def arrive_slot(eng):
        for i, ins in enumerate(b0.instructions):
            if isinstance(ins, mybir.InstDrain) and ins.engine == eng:
                return i + 1
        raise RuntimeError(f"no barrier drain found for {eng}")

    def after_release():
        last = None
        for i, ins in enumerate(b0.instructions):
            if isinstance(ins, mybir.InstEventSemaphore) and ins.engine == ET.Pool:
                last = i
        assert last is not None
        return last + 1

    for eng in (ET.SP, ET.Activation, ET.PE):
        if moved[eng]:
            pos = arrive_slot(eng)
            b0.instructions[pos:pos] = moved[eng]
    if moved[ET.Pool]:
        pos = after_release()
        b0.instructions[pos:pos] = moved[ET.Pool]


def _host_prep(x, means, bandwidths, weights, priors, fp8_sub):
    """Pack transposed GEMM operands; compute host-side affine terms."""
    import ml_dtypes

    x = np.asarray(x, dtype=np.float32)
    means = np.asarray(means, dtype=np.float32)

    bw = np.clip(np.asarray(bandwidths, dtype=np.float64), 0.001, 100.0)
    a = 1.0 / bw
    m_sq = np.einsum(
        "nd,nd->n", means.astype(np.float64), means.astype(np.float64)
    )
    w = np.asarray(weights, dtype=np.float64).reshape(C, K)
    log_w = (
        w
        - np.log(np.exp(w - w.max(1, keepdims=True)).sum(1, keepdims=True))
        - w.max(1, keepdims=True)
    ).reshape(N)
    pr = np.asarray(priors, dtype=np.float64)
    log_pri = pr - (np.log(np.exp(pr - pr.max()).sum()) + pr.max())
    cvec = (
        -0.5 * (D * LOG_2PI + D * np.log(bw) + m_sq * a)
        + log_w
        + np.repeat(log_pri, K)
    )
    ah = -0.5 * a
    xsq = np.einsum("bd,bd->b", x.astype(np.float64), x.astype(np.float64))

    # pack x into per-core, per-b-tile stripes [core, bt, p(row), bytes]:
    # fp8 subtile bytes then fp16 subtile bytes, matching the device bitcast
    nbt = BLOC // P
    ds = fp8_sub * P
    xt_t = x.T  # [D, B]
    mt_t = means.T * a  # [D, N]

    def pack_x(arr, dt):  # arr [d, B] -> [core, bt, p(row), sub*col] bytes
        sub = arr.shape[0] // P
        packed = np.ascontiguousarray(
            arr.astype(dt).reshape(sub, P, NCORES, nbt, P).transpose(2, 3, 1, 0, 4)
        )
        return packed.reshape(NCORES, nbt, P, -1).view(np.uint8)

    chunks = []
    if fp8_sub:
        chunks.append(pack_x(xt_t[:ds], ml_dtypes.float8_e4m3))
    if ds < D:
        chunks.append(pack_x(xt_t[ds:], np.float16))
    parts = {"xtc": np.concatenate(chunks, axis=3)}
    if fp8_sub:
        # [pair, half, p, s(2), 512]: element [c,h,p,s,j] = mt[(2c+s)*P+p,
        # h*512+j]
        m8 = mt_t[:ds].astype(ml_dtypes.float8_e4m3)
        m8 = m8.reshape(fp8_sub // 2, 2, P, 2, 512).transpose(0, 3, 2, 1, 4)
        parts["mt8t"] = np.ascontiguousarray(m8)
    if ds < D:
        m16 = mt_t[ds:].astype(np.float16)
        m16 = m16.reshape((D - ds) // P, P, 2, 512).transpose(0, 2, 1, 3)
        parts["mt16t"] = np.ascontiguousarray(m16)
    return parts, cvec, ah, xsq


def _host_finish(lp, cvec, ah, xsq):
    """lp: [B, N] fp16 device GEMM result -> [B, C] float32 log-mixture."""
    logp = lp.astype(np.float32)
    logp += cvec.astype(np.float32)[None, :]
    logp += xsq.astype(np.float32)[:, None] * ah.astype(np.float32)[None, :]
    v = logp.reshape(B, C, K)
    m = v.max(2)
    lse1 = m + np.log(np.exp(v - m[:, :, None]).sum(2, dtype=np.float32))
    z = lse1.max(1, keepdims=True)
    out = lse1 - (
        z + np.log(np.exp(lse1 - z).sum(1, keepdims=True, dtype=np.float32))
    )
    return out.astype(np.float32)


def _run(x, means, bandwidths, weights, priors, trace=False, cfg=None):
    from concourse.bass_utils import run_bass_kernel_spmd

    key = tuple(sorted((cfg or {}).items()))
    if key not in _CACHE:
        _CACHE[key] = _build_nc(cfg)
    nc = _CACHE[key]
    fp8_sub = (cfg or {}).get("fp8_sub", 6)

    parts, cvec, ah, xsq = _host_prep(
        x, means, bandwidths, weights, priors, fp8_sub
    )
    in_maps = [
        {
            k: np.ascontiguousarray(v[i]) if k.startswith("xt") else v
            for k, v in parts.items()
        }
        for i in range(NCORES)
    ]
    res = run_bass_kernel_spmd(nc, in_maps, core_ids=list(range(NCORES)), trace=trace)
    lp = np.concatenate([r["lp"] for r in res.results], axis=0)
    out = _host_finish(lp, cvec, ah, xsq)
    return out, res


def kernel(x, means, bandwidths, weights, priors):
    out, _ = _run(x, means, bandwidths, weights, priors, trace=False)
    return out
